# revision 32
# baseline (speedup 1.0000x reference)
"""GCN message-passing kernel for Trainium2, 8 NeuronCores (bf16 pipeline).

Math (reference): 3-layer GCN with symmetric normalization and self-loops,
then dot-product decode over label edge pairs.

Reformulations:
  * A_hat @ (x @ W) == (A_hat @ x) @ W: aggregate first, then one dense
    [128x128] @ W matmul per node block.
  * The symmetric norm factorizes: nrm_e = dinv[src] * dinv[dst].  The
    src factor is pre-folded into the gather tables (t_l = dinv * z_l,
    with t_0 = dinv * x folded on the host).  The dst factor commutes
    with the row-wise W matmul and with relu (dinv > 0), so it is applied
    as a per-partition ACT scale on the output block:
       t_{l+1} = relu(dinv^2 * (raw_agg @ W) + dinv * b)
       z_3     = dinv * (raw_agg @ W3) + b3
    The bias enters the PSUM as a rank-1 matmul sqrtdeg[node] (x) b[oc],
    which the scale turns back into the plain bias.
  * With norms folded away, the per-edge scatter indicator is a PURE 0/1
    one-hot, built with a single DVE is_equal tensor_tensor per block
    pair (broadcast access patterns), instead of one tensor_scalar per
    128-edge chunk.

Device mapping per core (owns 49 consecutive node blocks of 128):
  - each layer's gather table is split into two physical tensors: the
    A-region (every core's first 24 blocks, permuted to the front) and
    the B-region (the rest).  Each core computes its A blocks first and
    fires the A-half AllGather while its B blocks still compute; the
    next layer's A-region gathers depend only on that A-AllGather, so
    roughly half the (HBM-latency-bound) gather traffic overlaps the
    previous layer's B-half compute and B-AllGather.  Both regions are
    < 32768 rows, which also removes the int16 index-range split.
  - edges partitioned by dst block, sorted+chunked into 128-edge chunks;
    blocks processed in PAIRS so each dma_gather covers 2 blocks (one
    gather per region per pair, round-robined over 4 SWDGE queues).
  - PE accumulates psum[feat, node] += gathered[slot, feat].T @ onehot
  - per block: psum_z[node, oc] = sqrtdeg (x) b + aggT.T @ W, then ACT
    relu+scale and the bf16 slice is DMA'd out
Layer 3 output (64 feats) lands in a 128-wide padded table so decode
gather rows stay 256B-aligned; pad columns are never read.
Decode: labels bucketed by (a in A/B, b in A/B); per bucket dma_gather
of z3[a] and z3[b] rows, DVE multiply + reduce over the first 64 feats,
host inverse-permutes.
"""

import numpy as np
import ml_dtypes

P = 128
HALF = 32768
N_CORES = 8
BF16 = ml_dtypes.bfloat16


# ---------------------------------------------------------------- host prep

def _wrap16(flat_idx):
    """dma_gather idx layout: idx i at [i%16, i//16], replicated to 128 rows."""
    t = flat_idx.astype(np.int16).reshape(-1, 16).T  # [16, n/16]
    return np.tile(t, (8, 1))  # [128, n/16]


def row_perm(bpc, blocks_a):
    """Node id -> table row, with each core's A-half (first blocks_a
    blocks) laid out before all B-halves, so the two half-AllGathers
    produce contiguous regions: [r0A | r1A | ... | r7A | r0B | ... | r7B].
    """
    per = bpc * P
    mA = blocks_a * P
    v = np.arange(N_CORES * per, dtype=np.int64)
    c, m = v // per, v % per
    return np.where(m < mA, c * mA + m,
                    N_CORES * mA + c * (per - mA) + (m - mA))


def prepare_edges(edge_index, n_nodes, bpc, rowmap, rowsplit):
    """Build per-core gather/indicator streams, in block-pair order.

    Per pair (i, j) the chunk stream is [low_i, low_j, high_i, high_j]
    so one dma_gather covers both blocks' low (resp. high) chunks.
    Gather indices go through `rowmap` (the AB table permutation).
    Also returns the degree-derived per-node factors.
    """
    src = np.asarray(edge_index[0], dtype=np.int64)
    dst = np.asarray(edge_index[1], dtype=np.int64)
    deg = np.bincount(dst, minlength=n_nodes).astype(np.float64) + 1.0
    dinv = 1.0 / np.sqrt(deg)

    loops = np.arange(n_nodes, dtype=np.int64)
    esrc = rowmap[np.concatenate([src, loops])]
    edst = np.concatenate([dst, loops])

    blk = edst >> 7
    dnl = (edst & 127).astype(np.float32)
    n_blocks = N_CORES * bpc
    low = esrc < rowsplit

    order = np.lexsort((~low, blk))
    esrc, dnl_s, blk_s, low_s = esrc[order], dnl[order], blk[order], low[order]

    cnts = np.bincount(blk_s * 2 + (~low_s).astype(np.int64), minlength=2 * n_blocks)
    nlow = cnts[0::2]
    nhigh = cnts[1::2]

    nlow_2d = nlow.reshape(N_CORES, bpc)
    nhigh_2d = nhigh.reshape(N_CORES, bpc)
    cntl = np.ceil(nlow_2d / P).astype(np.int64).max(axis=0)
    cnth = np.ceil(nhigh_2d / P).astype(np.int64).max(axis=0)
    zero = (cntl + cnth) == 0
    cntl[zero] = 1

    C = int((cntl + cnth).sum())
    gidx = np.zeros((N_CORES, C * P), np.int64)
    # padding slots point at row 0 with dnl=255 (matches no iota value)
    gdnl = np.full((N_CORES, C * P), 255.0, np.float32)

    blk_starts = np.zeros(n_blocks + 1, np.int64)
    np.cumsum(nlow + nhigh, out=blk_starts[1:])

    pairs = [(i, i + 1 if i + 1 < bpc else None) for i in range(0, bpc, 2)]

    def fill(b, s0, n_take, cap, c, pos, off):
        take = min(n_take, cap)
        sl = slice(pos, pos + take)
        gidx[c, sl] = esrc[s0:s0 + take] - off
        gdnl[c, sl] = dnl_s[s0:s0 + take]

    for c in range(N_CORES):
        pos = 0
        for (i, j) in pairs:
            members = [i] if j is None else [i, j]
            for m in members:  # low chunks
                b = c * bpc + m
                fill(b, blk_starts[b], nlow[b], int(cntl[m]) * P, c, pos, 0)
                pos += int(cntl[m]) * P
            for m in members:  # B-region chunks
                b = c * bpc + m
                fill(b, blk_starts[b] + nlow[b], nhigh[b],
                     int(cnth[m]) * P, c, pos, rowsplit)
                pos += int(cnth[m]) * P
        assert pos == C * P

    eidx = np.stack([_wrap16(gidx[c]) for c in range(N_CORES)])
    ednl = np.ascontiguousarray(
        gdnl.reshape(N_CORES, C, P).transpose(0, 2, 1)).astype(BF16)
    return dict(eidx=eidx, ednl=ednl, cntl=cntl.astype(int),
                cnth=cnth.astype(int), C=C, pairs=pairs, dinv=dinv)


def prepare_labels(edge_label_index, n_label, rowmap, rowsplit):
    """Bucket labels by (a<HALF, b<HALF) per core, pad to 128 multiples."""
    a = rowmap[np.asarray(edge_label_index[0], dtype=np.int64)]
    b = rowmap[np.asarray(edge_label_index[1], dtype=np.int64)]
    per = n_label // N_CORES
    buckets_per_core = []
    for c in range(N_CORES):
        la = a[c * per:(c + 1) * per]
        lb = b[c * per:(c + 1) * per]
        lab = np.arange(c * per, (c + 1) * per)
        bid = (la >= rowsplit) * 2 + (lb >= rowsplit)
        buckets_per_core.append([(la[bid == k], lb[bid == k], lab[bid == k])
                                 for k in range(4)])
    tcnt = [max(int(np.ceil(len(buckets_per_core[c][k][0]) / P))
                for c in range(N_CORES)) for k in range(4)]
    T = sum(tcnt)
    aidx = np.zeros((N_CORES, T * P), np.int64)
    bidx = np.zeros((N_CORES, T * P), np.int64)
    labmap = np.full((N_CORES, T * P), -1, np.int64)
    for c in range(N_CORES):
        pos = 0
        for k in range(4):
            la, lb, lab = buckets_per_core[c][k]
            n = len(la)
            cap = tcnt[k] * P
            aidx[c, pos:pos + n] = la - (rowsplit if k >= 2 else 0)
            bidx[c, pos:pos + n] = lb - (rowsplit if k % 2 else 0)
            labmap[c, pos:pos + n] = lab
            pos += cap
    la_s = np.stack([_wrap16(aidx[c]) for c in range(N_CORES)])
    lb_s = np.stack([_wrap16(bidx[c]) for c in range(N_CORES)])
    return dict(la=la_s, lb=lb_s, tcnt=tcnt, T=T, labmap=labmap)


# ------------------------------------------------------------- device kernel

def build_bass(n_nodes, bpc, blocks_a, cntl, cnth, pairs, tcnt,
               in_c, hid_c, out_c):
    from concourse import bacc, bass, mybir
    from concourse.ap import AP
    import concourse.tile as tile

    NPAD = N_CORES * bpc * P
    rowsA = blocks_a * P            # per-core A-half rows
    rowsB = (bpc - blocks_a) * P
    npairs_a = blocks_a // 2
    CMAX = max(int(cntl[i]) + int(cnth[i])
               + (int(cntl[j]) + int(cnth[j]) if j is not None else 0)
               for (i, j) in pairs)
    C = int(sum(cntl) + sum(cnth))
    T = int(sum(tcnt))
    f32 = mybir.dt.float32
    bf16 = mybir.dt.bfloat16
    AF = mybir.ActivationFunctionType

    nc = bacc.Bacc("TRN2", target_bir_lowering=False, debug=False,
                   num_devices=N_CORES, num_swdge_queues=4)

    rsplit = N_CORES * rowsA
    xbA_d = nc.dram_tensor("xbA", [rsplit, in_c], bf16, kind="ExternalInput")
    xbB_d = nc.dram_tensor("xbB", [NPAD - rsplit, in_c], bf16,
                           kind="ExternalInput")
    w_d = [nc.dram_tensor(f"W{i+1}", s, bf16, kind="ExternalInput")
           for i, s in enumerate([[in_c, hid_c], [hid_c, hid_c], [hid_c, out_c]])]
    b_d = [nc.dram_tensor(f"b{i+1}", [s], bf16, kind="ExternalInput")
           for i, s in enumerate([hid_c, hid_c, out_c])]
    eidx_d = nc.dram_tensor("eidx", [P, C * P // 16], mybir.dt.int16,
                            kind="ExternalInput")
    ednl_d = nc.dram_tensor("ednl", [P, C], bf16, kind="ExternalInput")
    sdeg_d = nc.dram_tensor("sdeg", [1, bpc * P], bf16, kind="ExternalInput")
    dinv1_d = nc.dram_tensor("dinv1", [P, bpc], f32, kind="ExternalInput")
    dinv2_d = nc.dram_tensor("dinv2", [P, bpc], f32, kind="ExternalInput")
    la_d = nc.dram_tensor("la", [P, T * P // 16], mybir.dt.int16,
                          kind="ExternalInput")
    lb_d = nc.dram_tensor("lb", [P, T * P // 16], mybir.dt.int16,
                          kind="ExternalInput")
    out_d = nc.dram_tensor("out", [P, T], f32, kind="ExternalOutput")

    # internal DRAM: per-layer table slices (local, split A/B so the
    # A-half AllGather can fire while B-half blocks still compute) +
    # allgathered table (shared).  Layer-3 output is padded to 128 feats
    # (256B rows for decode gathers).
    zsA_d = [nc.dram_tensor(f"zsA{l}", [rowsA, hid_c], bf16, kind="Internal")
             for l in range(3)]
    zsB_d = [nc.dram_tensor(f"zsB{l}", [rowsB, hid_c], bf16, kind="Internal")
             for l in range(3)]
    zfA_d = [nc.dram_tensor(f"zfA{l}", [rsplit, hid_c], bf16, kind="Internal",
                            addr_space="Shared")
             for l in range(3)]
    zfB_d = [nc.dram_tensor(f"zfB{l}", [NPAD - rsplit, hid_c], bf16,
                            kind="Internal", addr_space="Shared")
             for l in range(3)]

    gq = [0]

    def next_q():
        q = gq[0]
        gq[0] = (q + 1) % 4
        return q

    def bcast_chunks(ap2d, cnt):
        """[P, n] AP -> [P, cnt, n] with chunk stride 0."""
        return AP(ap2d.tensor, ap2d.offset,
                  [list(ap2d.ap[0]), [0, cnt], list(ap2d.ap[1])])

    def bcast_inner(ap2d, n):
        """[P, cnt] AP -> [P, cnt, n] with inner stride 0."""
        return AP(ap2d.tensor, ap2d.offset,
                  [list(ap2d.ap[0]), list(ap2d.ap[1]), [0, n]])

    with tile.TileContext(nc) as tc:
        with (
            tc.tile_pool(name="consts", bufs=1) as cst,
            tc.tile_pool(name="gath", bufs=6) as gp,
            tc.tile_pool(name="indp", bufs=4) as ip,
            tc.tile_pool(name="dec", bufs=2) as dp,
            tc.tile_pool(name="work", bufs=6) as wp,
            tc.tile_pool(name="outp", bufs=4) as op,
            tc.tile_pool(name="psum", bufs=3, space="PSUM") as ps,
        ):
            # ---- constants and streams (resident whole kernel)
            iota32 = cst.tile([P, P], f32)
            nc.gpsimd.iota(iota32[:], pattern=[[1, P]], base=0,
                           channel_multiplier=0,
                           allow_small_or_imprecise_dtypes=True)
            iota = cst.tile([P, P], bf16)
            nc.vector.tensor_copy(out=iota[:], in_=iota32[:])

            eidx_sb = cst.tile([P, C * P // 16], mybir.dt.int16)
            ednl_sb = cst.tile([P, C], bf16)
            nc.sync.dma_start(eidx_sb[:], eidx_d[:, :])
            nc.sync.dma_start(ednl_sb[:], ednl_d[:, :])
            sdeg_sb = cst.tile([1, bpc * P], bf16)
            dinv1_sb = cst.tile([P, bpc], f32)
            dinv2_sb = cst.tile([P, bpc], f32)
            nc.sync.dma_start(sdeg_sb[:], sdeg_d[:, :])
            nc.sync.dma_start(dinv1_sb[:], dinv1_d[:, :])
            nc.sync.dma_start(dinv2_sb[:], dinv2_d[:, :])
            la_sb = cst.tile([P, T * P // 16], mybir.dt.int16)
            lb_sb = cst.tile([P, T * P // 16], mybir.dt.int16)
            nc.sync.dma_start(la_sb[:], la_d[:, :])
            nc.sync.dma_start(lb_sb[:], lb_d[:, :])

            w_sb = []
            bias_sb = []
            for l in range(3):
                wt = cst.tile([hid_c, out_c if l == 2 else hid_c], bf16)
                nc.sync.dma_start(wt[:], w_d[l][:, :])
                w_sb.append(wt)
                bt = cst.tile([1, out_c if l == 2 else hid_c], bf16)
                nc.sync.dma_start(bt[:], b_d[l][None, :])
                bias_sb.append(bt)

            # ---- 3 GCN layers
            for l in range(3):
                oc = out_c if l == 2 else hid_c
                tabA = xbA_d if l == 0 else zfA_d[l - 1]
                tabB = xbB_d if l == 0 else zfB_d[l - 1]
                lo_tab = tabA[:, :]
                hi_tab = tabB[:, :]

                chunk_base = 0
                for pi, (i, j) in enumerate(pairs):
                    members = [i] if j is None else [i, j]
                    cls = [int(cntl[m]) for m in members]
                    chs = [int(cnth[m]) for m in members]
                    ncl, nch = sum(cls), sum(chs)
                    cnt = ncl + nch
                    gt = gp.tile([P, cnt * in_c], bf16, tag="gath")
                    g3 = gt[:].rearrange("p (c f) -> p c f", c=cnt)
                    if ncl:
                        nc.gpsimd.dma_gather(
                            out_ap=g3[:, 0:ncl, :] if nch else g3,
                            in_ap=lo_tab,
                            idxs_ap=eidx_sb[:, chunk_base * 8:
                                            (chunk_base + ncl) * 8],
                            num_idxs=ncl * P, num_idxs_reg=ncl * P,
                            elem_size=in_c,
                            single_packet=False, queue_num=next_q())
                    if nch:
                        nc.gpsimd.dma_gather(
                            out_ap=g3[:, ncl:, :] if ncl else g3,
                            in_ap=hi_tab,
                            idxs_ap=eidx_sb[:, (chunk_base + ncl) * 8:
                                            (chunk_base + cnt) * 8],
                            num_idxs=nch * P, num_idxs_reg=nch * P,
                            elem_size=in_c,
                            single_packet=False, queue_num=next_q())

                    # one-hot indicator for the whole pair in one DVE op:
                    # ind[p, c*P + n] = (iota[n] == dnl[p, c])
                    ind_t = ip.tile([P, cnt * P], bf16, tag="ind")
                    it = ind_t[:]
                    dnl_ap = ednl_sb[:, chunk_base:chunk_base + cnt]
                    nc.vector.tensor_tensor(
                        out=it.rearrange("p (c n) -> p c n", c=cnt),
                        in0=bcast_chunks(iota[:], cnt),
                        in1=AP(dnl_ap.tensor, dnl_ap.offset,
                               [list(dnl_ap.ap[0]), [1, cnt], [0, P]]),
                        op=mybir.AluOpType.is_equal)

                    # per-block accumulation psums; chunk stream order is
                    # [low_0, low_1, high_0, high_1]
                    agg_ps = {m: ps.tile([P, P], f32, tag=f"agg{mi}",
                                         name=f"agg_ps{mi}", space="PSUM")
                              for mi, m in enumerate(members)}
                    segs = []
                    off = 0
                    for m, cl in zip(members, cls):
                        segs.append((m, off, cl))
                        off += cl
                    for m, ch in zip(members, chs):
                        segs.append((m, off, ch))
                        off += ch
                    first = {}
                    last = {}
                    for m, s0, n in segs:
                        if n == 0:
                            continue
                        if m not in first:
                            first[m] = s0
                        last[m] = s0 + n - 1
                    for m, s0, n in segs:
                        for k in range(s0, s0 + n):
                            nc.tensor.matmul(
                                out=agg_ps[m][:], lhsT=g3[:, k, :],
                                rhs=ind_t[:, k * P:(k + 1) * P],
                                start=(k == first[m]), stop=(k == last[m]))

                    for m in members:
                        aggT = wp.tile([P, P], bf16, tag="aggT")
                        nc.scalar.copy(out=aggT[:], in_=agg_ps[m][:])

                        z_ps = ps.tile([P, oc], f32, tag="z", bufs=2,
                                       space="PSUM")
                        nc.tensor.matmul(
                            out=z_ps[:],
                            lhsT=sdeg_sb[:, m * P:(m + 1) * P],
                            rhs=bias_sb[l][:], start=True, stop=False)
                        nc.tensor.matmul(out=z_ps[:], lhsT=aggT[:],
                                         rhs=w_sb[l][:],
                                         start=False, stop=True)

                        z_sb = op.tile([P, oc], bf16, tag="z_sb")
                        if l < 2:
                            nc.scalar.activation(
                                out=z_sb[:], in_=z_ps[:], func=AF.Relu,
                                scale=dinv2_sb[:, m:m + 1])
                        else:
                            nc.scalar.activation(
                                out=z_sb[:], in_=z_ps[:], func=AF.Identity,
                                scale=dinv1_sb[:, m:m + 1])
                        if m < blocks_a:
                            dst = zsA_d[l][m * P:(m + 1) * P, 0:oc]
                        else:
                            mm = m - blocks_a
                            dst = zsB_d[l][mm * P:(mm + 1) * P, 0:oc]
                        nc.sync.dma_start(dst, z_sb[:])
                    chunk_base += cnt

                    if pi == npairs_a - 1:
                        # A-half AllGather fires while B-half computes;
                        # next layer's A-region gathers depend only on it
                        nc.gpsimd.collective_compute(
                            "AllGather", mybir.AluOpType.bypass,
                            replica_groups=[list(range(N_CORES))],
                            ins=[zsA_d[l][:, :]], outs=[zfA_d[l][:, :]])

                nc.gpsimd.collective_compute(
                    "AllGather", mybir.AluOpType.bypass,
                    replica_groups=[list(range(N_CORES))],
                    ins=[zsB_d[l][:, :]], outs=[zfB_d[l][:, :]])

            # ---- decode
            tbase = 0
            res = cst.tile([P, T], f32)
            for k in range(4):
                tk = int(tcnt[k])
                if tk == 0:
                    continue
                a_tab = (zfB_d[2] if k >= 2 else zfA_d[2])[:, :]
                b_tab = (zfB_d[2] if k % 2 else zfA_d[2])[:, :]
                ga = dp.tile([P, tk * hid_c], bf16, tag="ga")
                gb = dp.tile([P, tk * hid_c], bf16, tag="gb")
                ga3 = ga[:].rearrange("p (c f) -> p c f", c=tk)
                gb3 = gb[:].rearrange("p (c f) -> p c f", c=tk)
                nc.gpsimd.dma_gather(
                    out_ap=ga3, in_ap=a_tab,
                    idxs_ap=la_sb[:, tbase * 8:(tbase + tk) * 8],
                    num_idxs=tk * P, num_idxs_reg=tk * P, elem_size=hid_c,
                    single_packet=False, queue_num=next_q())
                nc.gpsimd.dma_gather(
                    out_ap=gb3, in_ap=b_tab,
                    idxs_ap=lb_sb[:, tbase * 8:(tbase + tk) * 8],
                    num_idxs=tk * P, num_idxs_reg=tk * P, elem_size=hid_c,
                    single_packet=False, queue_num=next_q())
                nc.vector.tensor_mul(out=ga[:], in0=ga[:], in1=gb[:])
                nc.vector.tensor_reduce(
                    out=res[:, tbase:tbase + tk],
                    in_=ga3[:, :, 0:out_c],
                    axis=mybir.AxisListType.X, op=mybir.AluOpType.add)
                tbase += tk
            nc.sync.dma_start(out_d[:, :], res[:])

    nc.finalize()
    return nc


# ---------------------------------------------------------------- entry point

def kernel(x, W1, b1, W2, b2, W3, b3, edge_index, edge_label_index):
    from concourse.bass_utils import run_bass_kernel_spmd

    x = np.asarray(x, dtype=np.float32)
    n_nodes, in_c = x.shape
    hid_c = np.asarray(W2).shape[0]
    out_c = np.asarray(W3).shape[1]
    n_label = np.asarray(edge_label_index).shape[1]
    bpc = int(np.ceil(n_nodes / (N_CORES * P)))
    NPAD = N_CORES * bpc * P

    n_pairs = (bpc + 1) // 2
    blocks_a = 2 * (n_pairs // 2)
    rowmap = row_perm(bpc, blocks_a)
    rowsplit = N_CORES * blocks_a * P

    ed = prepare_edges(edge_index, n_nodes, bpc, rowmap, rowsplit)
    lb = prepare_labels(edge_label_index, n_label, rowmap, rowsplit)

    nc = build_bass(n_nodes, bpc, blocks_a, ed["cntl"], ed["cnth"],
                    ed["pairs"], lb["tcnt"], in_c, hid_c, out_c)

    dinv = np.zeros((NPAD,), np.float64)
    dinv[:n_nodes] = ed["dinv"]
    # src-side dinv pre-folded into the layer-0 table (permuted row order)
    xb = np.zeros((NPAD, in_c), BF16)
    xb[rowmap[:n_nodes]] = (x * ed["dinv"][:, None]).astype(BF16)
    sdeg = np.zeros((NPAD,), np.float64)
    sdeg[:n_nodes] = 1.0 / ed["dinv"]

    common = {
        "xbA": xb[:rowsplit],
        "xbB": xb[rowsplit:],
        "W1": np.ascontiguousarray(np.asarray(W1, np.float32).astype(BF16)),
        "W2": np.ascontiguousarray(np.asarray(W2, np.float32).astype(BF16)),
        "W3": np.ascontiguousarray(np.asarray(W3, np.float32).astype(BF16)),
        "b1": np.ascontiguousarray(np.asarray(b1, np.float32).astype(BF16)),
        "b2": np.ascontiguousarray(np.asarray(b2, np.float32).astype(BF16)),
        "b3": np.ascontiguousarray(np.asarray(b3, np.float32).astype(BF16)),
    }
    in_maps = []
    for c in range(N_CORES):
        lo = c * bpc * P
        hi = (c + 1) * bpc * P
        m = dict(common)
        m["eidx"] = np.ascontiguousarray(ed["eidx"][c])
        m["ednl"] = np.ascontiguousarray(ed["ednl"][c])
        m["sdeg"] = np.ascontiguousarray(
            sdeg[None, lo:hi].astype(BF16))
        m["dinv1"] = np.ascontiguousarray(
            dinv[lo:hi].reshape(bpc, P).T.astype(np.float32))
        m["dinv2"] = np.ascontiguousarray(
            (dinv[lo:hi] ** 2).reshape(bpc, P).T.astype(np.float32))
        m["la"] = np.ascontiguousarray(lb["la"][c])
        m["lb"] = np.ascontiguousarray(lb["lb"][c])
        in_maps.append(m)

    res = run_bass_kernel_spmd(nc, in_maps, core_ids=list(range(N_CORES)))

    out = np.zeros((n_label,), np.float32)
    for c in range(N_CORES):
        o = res.results[c]["out"]  # [P, T]
        flat = o.T.reshape(-1)
        lm = lb["labmap"][c]
        valid = lm >= 0
        out[lm[valid]] = flat[valid]
    return out
